# revision 74
# baseline (speedup 1.0000x reference)
"""Trainium2 Bass kernel: ViT transformer block with Convpass adapters.

Problem nn_CTrans_42133629173960 (dense_transformer, compute-bound).

Sharding: pure data-parallel over batch - 8 NeuronCores x 4 batches each,
no collectives. On-chip layout is feature-major ([channel, token]); the host
pre-transposes x/pos (and un-transposes the output).

v2: every large GEMM runs fp8e4 DoubleRow (2x K per PE pass):
  - QKV / V / proj / LN-stats / conv-down pair channel-tiles (ct, via
    host-prearranged [K,2,M] weight layouts); scores pair head-dims
    (d, d+64) with the other head's slot zeroed; A@V pairs key-token
    blocks (kt, kt+2) so the exp output lands pre-paired.
  - Weights are host-scaled x16 (fp8 subnormal avoidance); descales fold
    into the exp scale (1/256), the conv sigmoid scale, and the residual
    scalar_tensor_tensor ops. FFN weights are scaled x16 and descaled in
    the Gelu activation scale / final residual.
  - LayerNorm: raw-sum stats via fp8 ones-DR-matmuls; rstd =
    exp(-0.5*ln(var+eps)); rstd/mean broadcasts via K=1 PE matmul of a
    [1,1024] row; the normalize writes the fp8 pair tiles directly.
  - Softmax: denominator from a ones-column in V; one
    reciprocal_approx_fast per 4 heads; reciprocal rows broadcast via a
    K=4 indicator matmul; the normalize-multiply writes fp8 ac tiles.
  - exp runs on merged [128,1024] PSUM tiles (2 key blocks per call),
    matching the A@V pair layout; FFN Gelu runs on merged [128,1024]
    tiles when ff_b1 is zero.
  - x+pos adds and LN input casts run on the Pool engine (GpSimd).

Self-contained: hardcodes shapes from the problem spec.
"""

import numpy as np

import concourse.bass as bass
import concourse.tile as tile
from concourse import bacc, mybir
from concourse.bass_utils import run_bass_kernel_spmd

f32 = mybir.dt.float32
f32r = mybir.dt.float32r
bf16 = mybir.dt.bfloat16
f8 = mybir.dt.float8e4
DR = mybir.MatmulPerfMode.DoubleRow
AF = mybir.ActivationFunctionType
ALU = mybir.AluOpType

B, N, C = 32, 512, 512
H, DH = 8, 64
ADIM = 8
MLP = 4096
EPS = 1e-5
SCALE = DH ** -0.5
NCORES = 8
BPC = B // NCORES          # 4 batches per core
TOK = BPC * N              # 2048 tokens per core
P = 128
CT = C // P                # 4 channel tiles
CP = CT // 2               # 2 channel pair-tiles
NT = N // P                # 4 token sub-tiles per batch
MT1 = MLP // P             # 32 tiles of the FFN hidden dim
W1G = 4                    # ff_w1 resident groups (8 m-tiles each)
QSCALE = 1.702             # quick-gelu sigmoid scale
WS = 16.0                  # fp8 weight pre-scale (all GEMM weights)
IWS = 1.0 / WS


def _bias_tiles(nc, pool, dram_ap, n_tiles, name):
    """Load a [n_tiles*128, 1] DRAM vector as per-partition scalar tiles."""
    tiles = []
    for t in range(n_tiles):
        bt = pool.tile([P, 1], f32, name=f"{name}{t}")
        nc.sync.dma_start(bt[:], dram_ap[t * P:(t + 1) * P, :])
        tiles.append(bt)
    return tiles


def build(nc, nz, sim_gelu=False):
    """Emit the per-core program. nz: dict of which biases are nonzero."""
    xin = nc.dram_tensor("x", [C, TOK], f32, kind="ExternalInput").ap()
    pos = nc.dram_tensor("pos", [C, TOK], f32, kind="ExternalInput").ap()
    # fp8 pair layouts, host-prearranged (see make_in_maps)
    qkv_w = nc.dram_tensor("qkv_w", [CP, P, 2 * 3 * C], f8, kind="ExternalInput").ap()
    proj_w = nc.dram_tensor("proj_w", [CP, P, 2 * C], f8, kind="ExternalInput").ap()
    ff_w1 = nc.dram_tensor("ff_w1", [W1G, P, CT * 8 * P], f8, kind="ExternalInput").ap()
    ff_w2 = nc.dram_tensor("ff_w2", [W1G, P, 8 * C], f8, kind="ExternalInput").ap()
    cp_down_w = [nc.dram_tensor(f"cp{i}_down_w", [CP, P, 2 * 96], f8, kind="ExternalInput").ap() for i in (1, 2)]
    cp_conv_w = [nc.dram_tensor(f"cp{i}_conv_w", [96, 72], bf16, kind="ExternalInput").ap() for i in (1, 2)]
    cp_up_w = [nc.dram_tensor(f"cp{i}_up_w", [ADIM, C], bf16, kind="ExternalInput").ap() for i in (1, 2)]

    def opt_vec(name, length):
        if not nz.get(name, False):
            return None
        return nc.dram_tensor(name, [length, 1], f32, kind="ExternalInput").ap()

    proj_b = opt_vec("proj_b", C)
    ff_b1 = opt_vec("ff_b1", MLP)
    ff_b2 = opt_vec("ff_b2", C)
    cp_down_b = [opt_vec(f"cp{i}_down_b", ADIM) for i in (1, 2)]
    cp_conv_b = [opt_vec(f"cp{i}_conv_b", ADIM) for i in (1, 2)]
    cp_up_b = [opt_vec(f"cp{i}_up_b", C) for i in (1, 2)]
    ln_g = [opt_vec("ln1_g", C), opt_vec("ln2_g", C)]
    ln_b = [opt_vec("ln1_b", C), opt_vec("ln2_b", C)]

    out = nc.dram_tensor("out", [C, TOK], f32, kind="ExternalOutput").ap()

    with tile.TileContext(nc) as tc:
        _build_tc(nc, tc, dict(
            xin=xin, pos=pos, qkv_w=qkv_w, proj_w=proj_w, ff_w1=ff_w1,
            ff_w2=ff_w2, cp_down_w=cp_down_w, cp_conv_w=cp_conv_w,
            cp_up_w=cp_up_w, proj_b=proj_b, ff_b1=ff_b1, ff_b2=ff_b2,
            cp_down_b=cp_down_b, cp_conv_b=cp_conv_b, cp_up_b=cp_up_b,
            ln_g=ln_g, ln_b=ln_b, out=out), sim_gelu=sim_gelu)
    return nc


def _build_tc(nc, tc, t, sim_gelu=False):
    wdma = lambda eng_out, dram: nc.sync.dma_start(eng_out, dram)
    from contextlib import ExitStack

    bslc = lambda b: slice(b * N, (b + 1) * N)

    with ExitStack() as top:
        const = top.enter_context(tc.tile_pool(name="const", bufs=1))
        ones_f32 = const.tile([P, P], f32, name="ones_f32")
        nc.vector.memset(ones_f32[:], 1.0)
        zeros_f32 = const.tile([P, 800], f32, name="zeros_f32")
        nc.vector.memset(zeros_f32[:], 0.0)
        # fp8 ones for the LN stats DR matmuls (raw sums; 1/C applied later)
        ones8 = const.tile([P, 2, P], f8, name="ones8")
        nc.vector.memset(ones8[:].rearrange("p a b -> p (a b)"), 1.0)
        eps_t = const.tile([1, 1], f32, name="eps_t")
        nc.vector.memset(eps_t[:], EPS)
        # prewarm the ScalarE activation tables during the input-DMA wait,
        # ending with Ln resident (Square lives in every table set), so the
        # first LN chain pays one table load instead of two
        warm1 = const.tile([1, 1], f32, name="warm1")
        nc.scalar.activation(warm1[:], eps_t[:], AF.Square)
        nc.scalar.activation(warm1[:], eps_t[:], AF.Exp)
        nc.scalar.activation(warm1[:], eps_t[:], AF.Ln, bias=eps_t[0:1, :])

        bias_pool = top.enter_context(tc.tile_pool(name="biases", bufs=1))
        projb_sb = _bias_tiles(nc, bias_pool, t["proj_b"], CT, "projb") if t["proj_b"] is not None else None
        ffb1_sb = _bias_tiles(nc, bias_pool, t["ff_b1"], MT1, "ffb1") if t["ff_b1"] is not None else None
        ffb2_sb = _bias_tiles(nc, bias_pool, t["ff_b2"], CT, "ffb2") if t["ff_b2"] is not None else None
        lng_sb = [None, None]
        lnb_sb = [None, None]
        for i in range(2):
            if t["ln_g"][i] is not None:
                lng_sb[i] = _bias_tiles(nc, bias_pool, t["ln_g"][i], CT, f"lng{i}")
            if t["ln_b"][i] is not None:
                lnb_sb[i] = _bias_tiles(nc, bias_pool, t["ln_b"][i], CT, f"lnb{i}")
        cp_db = [None, None]
        cp_cb = [None, None]
        cp_upb = [None, None]
        for i in range(2):
            if t["cp_down_b"][i] is not None:
                db = bias_pool.tile([96, 1], f32, name=f"db_{i}")
                nc.vector.tensor_copy(db[:], zeros_f32[0:96, 0:1])
                for dxi in range(3):
                    nc.sync.dma_start(db[32 * dxi:32 * dxi + ADIM, :], t["cp_down_b"][i][:])
                cp_db[i] = db
            if t["cp_conv_b"][i] is not None:
                cb = bias_pool.tile([ADIM, 1], f32, name=f"cb_{i}")
                nc.sync.dma_start(cb[:], t["cp_conv_b"][i][:])
                cp_cb[i] = cb
            if t["cp_up_b"][i] is not None:
                cp_upb[i] = _bias_tiles(nc, bias_pool, t["cp_up_b"][i], CT, f"upb{i}")

        # convpass weights (small, persistent); all host-prearranged
        cpw_pool = top.enter_context(tc.tile_pool(name="cpw", bufs=1))
        dw96_sb = [[], []]
        w96_sb = [None, None]
        upw_sb = [None, None]
        for i in range(2):
            for cp in range(CP):
                dw96_sb[i].append(cpw_pool.tile([P, 2, 96], f8, name=f"dw96_{i}_{cp}"))
            w96_sb[i] = cpw_pool.tile([96, 9, ADIM], bf16, name=f"w96_{i}")
            upw_sb[i] = cpw_pool.tile([ADIM, C], bf16, name=f"upw_{i}")

        def load_cpw():
            for i in range(2):
                for cp in range(CP):
                    wdma(dw96_sb[i][cp][:].rearrange("p a b -> p (a b)"),
                         t["cp_down_w"][i][cp])
                wdma(w96_sb[i][:].rearrange("p a b -> p (a b)"), t["cp_conv_w"][i][:])
                wdma(upw_sb[i][:], t["cp_up_w"][i][:])

        # persistent activations; xT doubles as the residual carrier
        pool_xT = top.enter_context(tc.tile_pool(name="xT", bufs=1))
        xT = [pool_xT.tile([P, TOK], f32r, name=f"xT{ct}") for ct in range(CT)]
        x1T = xT
        # LN1 output, fp8 pair tiles (ct = 2*cp + j), consumed by QKV + conv1
        pool_h1 = top.enter_context(tc.tile_pool(name="h1", bufs=1))
        h1 = [pool_h1.tile([P, 2, TOK], f8, name=f"h1_{cp}") for cp in range(CP)]
        # LN2 output, same layout, consumed by FFN + conv2
        pool_h8 = top.enter_context(tc.tile_pool(name="h8", bufs=1))
        h8 = [pool_h8.tile([P, 2, TOK], f8, name=f"h8_{cp}") for cp in range(CP)]

        # ---- attention weights ----
        att_es = ExitStack()
        wpool = att_es.enter_context(tc.tile_pool(name="attw", bufs=1))
        qkvw_sb = [wpool.tile([P, 2, 3 * C], f8, name=f"qkvw{cp}") for cp in range(CP)]
        projw_sb = [wpool.tile([P, 2, C], f8, name=f"projw{cp}") for cp in range(CP)]

        # ---- LayerNorm. Raw sums S1/S2 via fp8 ones-DR-matmuls; the
        #      scalar chain runs on [1,N] rows; rstd/mr broadcast via a
        #      single K=1 PE matmul into a [128,1024] PSUM tile; the
        #      normalize writes the fp8 pair tiles (h1 or h8) directly.
        def layer_norm_multi(src, dst8, which, bs, lnps, lnsb, psname="ln_s"):
            """LayerNorm for a list of batches. The Ln and Exp rows of every
            batch run back-to-back (function-batched) so ScalarE loads each
            activation table once per call instead of once per batch."""
            rb = len(bs)
            per = []
            for b in bs:
                xb = []
                for cp in range(CP):
                    xb_t = lnsb.tile([P, 2, N], f8, name=f"ln_xb{cp}", bufs=2)
                    sq_t = lnsb.tile([P, 2, N], f8, name=f"ln_sq{cp}", bufs=2)
                    for j in range(2):
                        ct = 2 * cp + j
                        nc.vector.tensor_copy(xb_t[:, j, :], src[ct][:, bslc(b)])
                        nc.scalar.activation(sq_t[:, j, :], src[ct][:, bslc(b)],
                                             AF.Square)
                    xb.append((xb_t, sq_t))
                s1 = lnps.tile([P, N], f32, name=psname)
                for cp in range(CP):
                    nc.tensor.matmul(s1[:], ones8[:], xb[cp][0][:],
                                     start=(cp == 0), stop=(cp == CP - 1),
                                     perf_mode=DR)
                s2 = lnps.tile([P, N], f32, name=psname)
                for cp in range(CP):
                    nc.tensor.matmul(s2[:], ones8[:], xb[cp][1][:],
                                     start=(cp == 0), stop=(cp == CP - 1),
                                     perf_mode=DR)
                # consume the PSUM rows right away (Square/Copy are in every
                # activation table) so the stats ring can cycle
                msq1 = lnsb.tile([1, N], f32, name="ln_msq1", bufs=1)
                nc.scalar.activation(msq1[:], s1[0:1, :], AF.Square, scale=1.0 / C)
                if rb > 1:
                    # free the PSUM row before the function-batched passes
                    s1row = lnsb.tile([1, N], f32, name="ln_s1r", bufs=rb)
                    nc.scalar.activation(s1row[:], s1[0:1, :], AF.Copy,
                                         scale=1.0 / C)
                else:
                    s1row = None
                var1 = lnsb.tile([1, N], f32, name="ln_var1", bufs=rb)
                nc.vector.scalar_tensor_tensor(var1[:], s2[0:1, :], 1.0 / C, msq1[:],
                                               op0=ALU.mult, op1=ALU.subtract)
                per.append({"s1row": s1row, "s1": s1, "var1": var1})
            for p in per:
                lnv1 = lnsb.tile([1, N], f32, name="ln_lnv1", bufs=rb)
                nc.scalar.activation(lnv1[:], p["var1"][:], AF.Ln, bias=eps_t[0:1, :])
                p["lnv1"] = lnv1
            for p in per:
                rstd1f = lnsb.tile([1, N], f32, name="ln_rstd1", bufs=rb)
                nc.scalar.activation(rstd1f[:], p["lnv1"][:], AF.Exp, scale=-0.5)
                p["rstd"] = rstd1f
            for b, p in zip(bs, per):
                mr1f = lnsb.tile([1, N], f32, name="ln_mr1", bufs=1)
                if p["s1row"] is not None:
                    nc.vector.tensor_mul(mr1f[:], p["s1row"][:], p["rstd"][:])
                else:
                    nc.vector.scalar_tensor_tensor(mr1f[:], p["s1"][0:1, :], 1.0 / C,
                                                   p["rstd"][:],
                                                   op0=ALU.mult, op1=ALU.mult)
                # broadcast on the Pool engine so the PE stream never waits
                rstd_bc = lnsb.tile([P, N], f32, name="ln_rstd_bc", bufs=2)
                nc.gpsimd.partition_broadcast(rstd_bc[:], p["rstd"][:])
                mr_bc = lnsb.tile([P, N], f32, name="ln_mr_bc", bufs=2)
                nc.gpsimd.partition_broadcast(mr_bc[:], mr1f[:])
                for cp in range(CP):
                    for j in range(2):
                        ct = 2 * cp + j
                        if lng_sb[which] is not None or lnb_sb[which] is not None:
                            tmp = lnsb.tile([P, N], f32, name="ln_tmp", bufs=2)
                            nc.vector.tensor_mul(tmp[:], src[ct][:, bslc(b)],
                                                 rstd_bc[:])
                            hts = lnsb.tile([P, N], f32, name="ln_hts", bufs=2)
                            nc.vector.tensor_sub(hts[:], tmp[:], mr_bc[:])
                            g = lng_sb[which][ct][:] if lng_sb[which] is not None else 1.0
                            bb = lnb_sb[which][ct][:] if lnb_sb[which] is not None else 0.0
                            nc.vector.tensor_scalar(dst8[cp][:, j, bslc(b)], hts[:],
                                                    g, bb, op0=ALU.mult, op1=ALU.add)
                        else:
                            tmp = lnsb.tile([P, N], f32, name="ln_tmp", bufs=2)
                            nc.vector.tensor_mul(tmp[:], src[ct][:, bslc(b)],
                                                 rstd_bc[:])
                            nc.vector.tensor_sub(dst8[cp][:, j, bslc(b)], tmp[:],
                                                 mr_bc[:])

        # ---- Convpass (i=0,1): down-GEMM is fp8-DR from the h pair tiles;
        #      the 3x3x3 conv + up-GEMM stay bf16 (tiny).
        def convpass_all(i, src8, fold, es, pre_batch=None, up_bufs=2,
                         pre_shift=False, tail_hook=None, csb=None, zero_ims=True):
            if csb is None:
                csb = es.enter_context(tc.tile_pool(name=f"cp{i}sb", bufs=1))
            dnps = es.enter_context(tc.tile_pool(name=f"cp{i}dn", bufs=2, space="PSUM"))
            cvps = es.enter_context(tc.tile_pool(name=f"cp{i}cv", bufs=2, space="PSUM"))
            upps = es.enter_context(tc.tile_pool(name=f"cp{i}up", bufs=up_bufs, space="PSUM"))
            ims, pts = [], []
            for b in range(BPC):
                if pre_batch is not None and not pre_shift:
                    pre_batch(b)
                d_ps = dnps.tile([96, N], f32, name="cp_dps")
                for cp in range(CP):
                    nc.tensor.matmul(d_ps[:], dw96_sb[i][cp][:], src8[cp][:, :, bslc(b)],
                                     start=(cp == 0), stop=(cp == CP - 1), perf_mode=DR)
                # d_ps carries x16 from the weight scale; descale in the
                # sigmoid scale and the qgelu multiply
                if cp_db[i] is not None:
                    dz = csb.tile([96, N], f32, name="cp_dz", bufs=4)
                    nc.vector.scalar_tensor_tensor(dz[:], d_ps[:], IWS, cp_db[i][:],
                                                   op0=ALU.mult, op1=ALU.add)
                    sg = csb.tile([96, N], f32, name="cp_sg", bufs=4)
                    nc.scalar.activation(sg[:], dz[:], AF.Sigmoid, scale=QSCALE)
                    d96 = csb.tile([96, N], bf16, name="cp_d96", bufs=4)
                    nc.vector.tensor_mul(d96[:], dz[:], sg[:])
                else:
                    sg = csb.tile([96, N], f32, name="cp_sg", bufs=4)
                    nc.scalar.activation(sg[:], d_ps[:], AF.Sigmoid, scale=QSCALE * IWS)
                    d96 = csb.tile([96, N], bf16, name="cp_d96", bufs=4)
                    nc.vector.scalar_tensor_tensor(d96[:], d_ps[:], IWS, sg[:],
                                                   op0=ALU.mult, op1=ALU.mult)
                im96 = csb.tile([96, 10, 10, 8], bf16, name="cp_im96", bufs=4)
                if zero_ims:  # conv2 reuses conv1's tiles; pads stay zero
                    nc.vector.tensor_copy(im96[:].rearrange("p a b c -> p (a b c)"),
                                          zeros_f32[0:96, 0:800])
                dv = d96[:].rearrange("p (z y x) -> p z y x", z=8, y=8)
                nc.vector.tensor_copy(im96[0:8, 1:9, 1:9, 1:8], dv[0:8, :, :, 0:7])
                nc.vector.tensor_copy(im96[32:40, 1:9, 1:9, 0:8], dv[32:40, :, :, 0:8])
                nc.vector.tensor_copy(im96[64:72, 1:9, 1:9, 0:7], dv[64:72, :, :, 1:8])
                ims.append(im96)
                if pre_batch is not None and pre_shift and b + 1 < BPC:
                    pre_batch(b + 1)
            for b in range(BPC):
                cv_ps = cvps.tile([ADIM, N], f32, name="cp_cvps")
                cv_view = cv_ps[:].rearrange("p (z y x) -> p z y x", z=8, y=8)
                for tap in range(9):
                    dzz, dyy = tap // 3, tap % 3
                    nc.tensor.matmul(cv_view, w96_sb[i][:, tap, :],
                                     ims[b][0:96, dzz:dzz + 8, dyy:dyy + 8, 0:8],
                                     start=(tap == 0), stop=(tap == 8))
                if cp_cb[i] is not None:
                    cz = csb.tile([ADIM, N], f32, name="cp_cz", bufs=4)
                    nc.vector.tensor_scalar_add(cz[:], cv_ps[:], cp_cb[i][:])
                    c_in = cz
                else:
                    c_in = cv_ps
                sg2 = csb.tile([ADIM, N], f32, name="cp_sg2", bufs=4)
                nc.scalar.activation(sg2[:], c_in[:], AF.Sigmoid, scale=QSCALE)
                pt = csb.tile([ADIM, N], bf16, name="cp_pt", bufs=4)
                nc.vector.tensor_mul(pt[:], c_in[:], sg2[:])
                pts.append(pt)
            for b in range(BPC):
                for ct in range(CT):
                    up_ps = upps.tile([P, N], f32, name="cp_upps")
                    nc.tensor.matmul(up_ps[:], upw_sb[i][:, ct * P:(ct + 1) * P],
                                     pts[b][:], start=True, stop=True)
                    fold(b, ct, up_ps)
                if tail_hook is not None:
                    tail_hook(b)

        # ---- Phase 2: attention ----
        with ExitStack() as esw:
            q_pool = esw.enter_context(tc.tile_pool(name="q8sb", bufs=2))
            k_pool = esw.enter_context(tc.tile_pool(name="k8sb", bufs=2))
            v_pool = esw.enter_context(tc.tile_pool(name="vsb", bufs=1))
            e_pool = esw.enter_context(tc.tile_pool(name="esb", bufs=1))
            a_pool = esw.enter_context(tc.tile_pool(name="acsb", bufs=1))
            n_pool = esw.enter_context(tc.tile_pool(name="nsb", bufs=2))

            qkps = esw.enter_context(tc.tile_pool(name="qkps", bufs=2, space="PSUM"))
            scps = esw.enter_context(tc.tile_pool(name="scps", bufs=2, space="PSUM"))
            avps = esw.enter_context(tc.tile_pool(name="avps", bufs=4, space="PSUM"))
            p0sb = esw.enter_context(tc.tile_pool(name="p0", bufs=6))
            ln1sb = esw.enter_context(tc.tile_pool(name="ln0sb", bufs=3))

            def phase0_ln1(b):
                # load+add batch b on the Pool engine, then LN1(b)
                for ct in range(CT):
                    xt = p0sb.tile([P, N], f32, name="xt_in")
                    pt = p0sb.tile([P, N], f32, name="pt_in")
                    nc.sync.dma_start(xt[:], t["xin"][ct * P:(ct + 1) * P, bslc(b)])
                    nc.sync.dma_start(pt[:], t["pos"][ct * P:(ct + 1) * P, bslc(b)])
                    nc.vector.tensor_add(xT[ct][:, bslc(b)], xt[:], pt[:])
                if b == 0:
                    for cp in range(CP):
                        wdma(qkvw_sb[cp][:].rearrange("p a b -> p (a b)"),
                             t["qkv_w"][cp])
                    for cp in range(CP):
                        wdma(projw_sb[cp][:].rearrange("p a b -> p (a b)"),
                             t["proj_w"][cp])
                layer_norm_multi(xT, h1, 0, [b], qkps, ln1sb, psname="qk_ps")

            def emit_qkv(b):
                # Q/K m-tiles (mt 0..3 Q, 4..7 K), V s-tiles. fp8-DR over cp.
                q8, k8, v8 = [], [], []
                for mt in range(8):
                    qk_ps = qkps.tile([P, N], f32, name="qk_ps")
                    for cp in range(CP):
                        nc.tensor.matmul(qk_ps[:],
                                         qkvw_sb[cp][:, :, mt * P:(mt + 1) * P],
                                         h1[cp][:, :, bslc(b)],
                                         start=(cp == 0), stop=(cp == CP - 1),
                                         perf_mode=DR)
                    if mt < 4:
                        # Q pair tiles: head h = 2*mt + hh; pair dim (d, d+64)
                        # of the m-tile; the other head's slot stays zero.
                        # The copies run on ScalarE to relieve the DVE.
                        for hh in range(2):
                            qp = q_pool.tile([DH, 2, N], f8, name=f"q8_{mt}_{hh}")
                            if b < 2:
                                nc.vector.tensor_copy(
                                    qp[:, 1 - hh, :], zeros_f32[0:DH, 0:N])
                            nc.scalar.activation(
                                qp[:, hh, :], qk_ps[hh * DH:(hh + 1) * DH, :],
                                AF.Copy)
                            q8.append(qp)
                    else:
                        kp = k_pool.tile([DH, 2, N], f8, name=f"k8_{mt}")
                        nc.vector.tensor_copy(kp[:, 0, :], qk_ps[0:DH, :])
                        nc.vector.tensor_copy(kp[:, 1, :], qk_ps[DH:P, :])
                        k8.append(kp)
                for s in range(NT):
                    v_ps = qkps.tile([P, C], f32, name="qk_ps")
                    for cp in range(CP):
                        nc.tensor.matmul(v_ps[:], h1[cp][:, :, b * N + s * P: b * N + (s + 1) * P],
                                         qkvw_sb[cp][:, :, 2 * C:3 * C],
                                         start=(cp == 0), stop=(cp == CP - 1),
                                         perf_mode=DR)
                    # v8 pair tiles: pair dim = key-block (s, s+2); per-head
                    # stride 128: col DH holds the softmax-denominator ones
                    pi, jj = s % 2, s // 2
                    if jj == 0:
                        v_t = v_pool.tile([P, 2, H * P], f8, name=f"v8_{pi}")
                        v8.append(v_t)
                    else:
                        v_t = v8[pi]
                    vv = v_t[:, jj, :].rearrange("p (h e) -> p h e", h=H)
                    nc.vector.tensor_copy(
                        vv[:, :, 0:DH],
                        v_ps[:].rearrange("p (h d) -> p h d", h=H))
                    if b == 0:
                        nc.vector.tensor_copy(
                            vv[:, :, DH:DH + 1],
                            ones_f32[:, 0:H].rearrange("p (h o) -> p h o", o=1))
                        nc.vector.tensor_copy(
                            vv[:, :, DH + 1:P],
                            zeros_f32[:, 0:H * (P - DH - 1)].rearrange("p (h o) -> p h o", h=H))
                return q8, k8, v8

            def emit_scores(q8, k8, g):
                # heads 2g, 2g+1; exp writes slot jj of the paired e8 tile
                # (pair = key blocks (0,2) / (1,3)) straight from each score
                # tile, so A@V consumes a ready DR operand.
                e8 = {}
                for hh in range(2):
                    for pi in range(2):
                        e_t = e_pool.tile([P, 2, N], f8, name=f"e8_{g}_{hh}_{pi}")
                        for jj in range(2):
                            kt = pi + 2 * jj
                            sc_ps = scps.tile([P, N], f32, name="sc_ps")
                            nc.tensor.matmul(sc_ps[:],
                                             k8[g][:, :, kt * P:(kt + 1) * P],
                                             q8[2 * g + hh][:],
                                             start=True, stop=True, perf_mode=DR)
                            nc.scalar.activation(e_t[:, jj, :], sc_ps[:],
                                                 AF.Exp, scale=SCALE / (WS * WS))
                        e8[(hh, pi)] = e_t
                return e8

            def emit_avmm(v8, g, e8, den4, dk):
                # A@V for heads 2g, 2g+1: 2 DR matmuls per head (pair blocks);
                # denominator rows land 32 apart in the half-batch den4 tile
                avs = []
                for hh in range(2):
                    h = 2 * g + hh
                    av_ps = avps.tile([P, N], f32, name="av_ps")
                    for pi in range(2):
                        nc.tensor.matmul(av_ps[:],
                                         v8[pi][:, :, h * P:(h + 1) * P],
                                         e8[(hh, pi)][:],
                                         start=(pi == 0), stop=(pi == 1),
                                         perf_mode=DR)
                    avs.append(av_ps)
                    k = dk + hh
                    nc.vector.tensor_copy(den4[32 * k:32 * k + 1, :],
                                          av_ps[DH:DH + 1, :])
                return avs

            def emit_norm(ac8, gp, den4, avs):
                # one reciprocal_approx_fast per 4 heads; Pool-engine
                # broadcasts keep the PE stream free; the multiply writes the
                # fp8 ac pair tiles (x16 carried from V).
                rcp4 = n_pool.tile([97, N], f32, name=f"rcp{gp}")
                nc.vector.reciprocal_approx_fast(rcp4[:], den4[:])
                for k in range(4):
                    h = 4 * gp + k
                    ct = h // 2
                    cp, j = ct // 2, ct % 2
                    if k == 0:
                        rsrc = rcp4[0:1, :]
                    else:
                        r1 = n_pool.tile([1, N], f32, name=f"r1_{k}")
                        nc.vector.tensor_copy(r1[:], rcp4[32 * k:32 * k + 1, :])
                        rsrc = r1[:]
                    bch = n_pool.tile([DH, N], f32, name=f"bch{k}")
                    nc.gpsimd.partition_broadcast(bch[:], rsrc)
                    nc.vector.tensor_mul(
                        ac8[cp][(h % 2) * DH:(h % 2) * DH + DH, j, :],
                        avs[k][0:DH, :],
                        bch[:])

            def emit_proj(b, ac8):
                for ct in range(CT):
                    pr_ps = avps.tile([P, N], f32, name="av_ps")
                    for cp in range(CP):
                        nc.tensor.matmul(pr_ps[:], projw_sb[cp][:, :, ct * P:(ct + 1) * P],
                                         ac8[cp][:], start=(cp == 0), stop=(cp == CP - 1),
                                         perf_mode=DR)
                    # pr_ps carries x256 (ac x16, proj_w x16)
                    if projb_sb is not None:
                        prb = n_pool.tile([P, N], f32, name="prb")
                        nc.vector.scalar_tensor_tensor(prb[:], pr_ps[:], 1.0 / (WS * WS),
                                                       projb_sb[ct][:],
                                                       op0=ALU.mult, op1=ALU.add)
                        nc.vector.tensor_add(x1T[ct][:, bslc(b)], xT[ct][:, bslc(b)], prb[:])
                    else:
                        nc.vector.scalar_tensor_tensor(
                            x1T[ct][:, bslc(b)], pr_ps[:], 1.0 / (WS * WS),
                            xT[ct][:, bslc(b)], op0=ALU.mult, op1=ALU.add)

            for _ in range(48):
                wt = qkps.tile([P, N], f32, name="qk_ps")
                nc.tensor.matmul(wt[:, 0:P], ones8[:], ones8[:],
                                 start=True, stop=True, perf_mode=DR)
            phase0_ln1(0)
            qkv_cur = emit_qkv(0)
            for b in range(BPC):
                q8, k8, v8 = qkv_cur
                ac8 = [a_pool.tile([P, 2, N], f8, name=f"ac8_{cp}") for cp in range(CP)]
                # software-pipelined by half-batch: scores/exp of the next
                # pair of head-groups are emitted before the A@V of the
                # current pair, so the in-order PE always has score matmuls
                # to run while ScalarE exps and the DVE normalizes.
                # LN1(b+1) is emitted FIRST: its two Ln/Exp table loads drain
                # from the Scalar queue before this batch's exp stream, which
                # then runs with the Exp table resident end-to-end.
                if b + 1 < BPC:
                    phase0_ln1(b + 1)
                e0 = emit_scores(q8, k8, 0)
                e1 = emit_scores(q8, k8, 1)
                den_a = n_pool.tile([97, N], f32, name="den_a")
                avs_a = emit_avmm(v8, 0, e0, den_a, 0)
                avs_a += emit_avmm(v8, 1, e1, den_a, 2)
                e2 = emit_scores(q8, k8, 2)
                e3 = emit_scores(q8, k8, 3)
                emit_norm(ac8, 0, den_a, avs_a)
                den_b = n_pool.tile([97, N], f32, name="den_b")
                avs_b = emit_avmm(v8, 2, e2, den_b, 0)
                avs_b += emit_avmm(v8, 3, e3, den_b, 2)
                if b + 1 < BPC:
                    qkv_cur = emit_qkv(b + 1)
                emit_norm(ac8, 1, den_b, avs_b)
                emit_proj(b, ac8)
        att_es.close()
        load_cpw()

        # ---- FFN weights: resident, loaded during the convpass1 window ----
        ffw_es = ExitStack()
        ffw_pool = ffw_es.enter_context(tc.tile_pool(name="ffw", bufs=1))
        w1_res = []
        w2_res = []
        for g in range(W1G):
            w1t = ffw_pool.tile([P, 2, 2, 8, P], f8, name=f"w1_{g}")
            wdma(w1t[:].rearrange("p a b c d -> p (a b c d)"), t["ff_w1"][g])
            w1_res.append(w1t)
        for g in range(W1G):
            w2t = ffw_pool.tile([P, 4, 2, C], f8, name=f"w2_{g}")
            wdma(w2t[:].rearrange("p a b c -> p (a b c)"), t["ff_w2"][g])
            w2_res.append(w2t)

        def w1_ap(mt, cp):
            return w1_res[mt // 8][:, cp, :, mt % 8, :]

        def w2_ap(mtp, ct):
            return w2_res[(2 * mtp) // 8][:, mtp % 4, :, ct * P:(ct + 1) * P]

        # ---- Phase 2b/4: convpass1 then convpass2. LN2 is software-
        # pipelined one batch ahead of conv2: LN2(0) runs under conv1's
        # fold stage (its PSUM pool spans both conv windows: 6+2=8 banks),
        # LN2(b+1) under conv2's stage-1(b), so conv2's down-matmuls never
        # wait on the LN2 chain.
        with ExitStack() as escln:
            lnps2 = escln.enter_context(tc.tile_pool(name="ln2ps", bufs=2, space="PSUM"))
            lnsb2 = escln.enter_context(tc.tile_pool(name="ln2sb", bufs=3))
            csb_sh = escln.enter_context(tc.tile_pool(name="cpsb", bufs=1))

            def ln2_pre(b):
                layer_norm_multi(x1T, h8, 1, [b], lnps2, lnsb2, psname="ln2_ps")

            with ExitStack() as escp1:
                def fold1(b, ct, up_ps):
                    if cp_upb[0] is not None:
                        ub = escp1_sb.tile([P, N], f32, name="upb_t", bufs=2)
                        nc.vector.tensor_scalar_add(ub[:], up_ps[:], cp_upb[0][ct][:])
                        nc.vector.tensor_add(x1T[ct][:, bslc(b)], x1T[ct][:, bslc(b)], ub[:])
                    else:
                        nc.vector.tensor_add(x1T[ct][:, bslc(b)], x1T[ct][:, bslc(b)], up_ps[:])
                escp1_sb = escp1.enter_context(tc.tile_pool(name="cp1fold", bufs=1))
                convpass_all(0, h1, fold1, escp1, csb=csb_sh,
                             tail_hook=lambda b: ln2_pre(0) if b == 0 else None)

            with ExitStack() as escp2:
                def fold2(b, ct, up_ps):
                    if cp_upb[1] is not None:
                        ub = escp2_sb.tile([P, N], f32, name="upb2_t", bufs=2)
                        nc.vector.tensor_scalar_add(ub[:], up_ps[:], cp_upb[1][ct][:])
                        nc.vector.tensor_add(x1T[ct][:, bslc(b)], x1T[ct][:, bslc(b)], ub[:])
                    else:
                        nc.vector.tensor_add(x1T[ct][:, bslc(b)], x1T[ct][:, bslc(b)], up_ps[:])
                escp2_sb = escp2.enter_context(tc.tile_pool(name="cp2fold", bufs=1))
                convpass_all(1, h8, fold2, escp2, pre_batch=ln2_pre,
                             pre_shift=True, csb=csb_sh, zero_ims=False)

        # ---- Phase 5: fused FFN per batch + residual + store.
        # f1 runs on merged [128,1024] PSUM tiles (one Gelu per m-tile pair
        # when ff_b1 is zero); weights carry x16, descaled in the Gelu scale
        # and the final residual.
        with tc.tile_pool(name="gmsb", bufs=3) as gmsb, \
             tc.tile_pool(name="outsb", bufs=4) as outsb, \
             tc.tile_pool(name="f1ps", bufs=2, space="PSUM") as f1ps, \
             tc.tile_pool(name="f2ps", bufs=1, space="PSUM") as f2ps:
            for b in range(BPC):
                f2acc = [f2ps.tile([P, N], f32, name=f"f2acc{ct}") for ct in range(CT)]

                def emit_f2(mtp, g8t, f2acc=f2acc):
                    for ct in range(CT):
                        nc.tensor.matmul(f2acc[ct][:], w2_ap(mtp, ct), g8t[:],
                                         start=(mtp == 0), stop=(mtp == MT1 // 2 - 1),
                                         perf_mode=DR)

                prev_g8 = None
                for mtp in range(MT1 // 2):
                    g8t = gmsb.tile([P, 2, N], f8, name="g8")
                    f1_ps = f1ps.tile([P, 2 * N], f32, name="f1_ps")
                    for jj in range(2):
                        mt = 2 * mtp + jj
                        for cp in range(CP):
                            nc.tensor.matmul(f1_ps[:, jj * N:(jj + 1) * N],
                                             w1_ap(mt, cp),
                                             h8[cp][:, :, bslc(b)],
                                             start=(cp == 0), stop=(cp == 1),
                                             perf_mode=DR)
                    if sim_gelu:
                        for jj in range(2):
                            fsg = gmsb.tile([P, N], f32, name="fsg")
                            nc.scalar.activation(fsg[:], f1_ps[:, jj * N:(jj + 1) * N],
                                                 AF.Sigmoid, scale=QSCALE * IWS)
                            nc.vector.scalar_tensor_tensor(
                                g8t[:, jj, :], f1_ps[:, jj * N:(jj + 1) * N], IWS,
                                fsg[:], op0=ALU.mult, op1=ALU.mult)
                    elif ffb1_sb is not None:
                        for jj in range(2):
                            mt = 2 * mtp + jj
                            nc.scalar.activation(g8t[:, jj, :],
                                                 f1_ps[:, jj * N:(jj + 1) * N],
                                                 AF.Gelu, bias=ffb1_sb[mt][:],
                                                 scale=IWS)
                    else:
                        nc.scalar.activation(g8t[:].rearrange("p a b -> p (a b)"),
                                             f1_ps[:], AF.Gelu, scale=IWS)
                    if prev_g8 is not None:
                        emit_f2(mtp - 1, prev_g8)
                    prev_g8 = g8t
                emit_f2(MT1 // 2 - 1, prev_g8)

                for ct in range(CT):
                    ofm = outsb.tile([P, N], f32, name="ofm")
                    if ffb2_sb is not None:
                        f2b = outsb.tile([P, N], f32, name="f2b")
                        nc.vector.scalar_tensor_tensor(f2b[:], f2acc[ct][:], IWS,
                                                       ffb2_sb[ct][:],
                                                       op0=ALU.mult, op1=ALU.add)
                        nc.vector.tensor_add(ofm[:], x1T[ct][:, bslc(b)], f2b[:])
                    else:
                        nc.vector.scalar_tensor_tensor(
                            ofm[:], f2acc[ct][:], IWS, x1T[ct][:, bslc(b)],
                            op0=ALU.mult, op1=ALU.add)
                    nc.sync.dma_start(
                        t["out"][ct * P:(ct + 1) * P, bslc(b)], ofm[:])
        ffw_es.close()


_CACHE = {}


def _get_compiled(nz_key, nz):
    if nz_key not in _CACHE:
        nc = bacc.Bacc("TRN2", target_bir_lowering=False, debug=False,
                       num_devices=NCORES)
        build(nc, nz)
        nc.compile()
        _CACHE[nz_key] = nc
    return _CACHE[nz_key]


def input_flags(inputs):
    nz = {}
    vec_names = ["proj_b", "ff_b1", "ff_b2", "cp1_down_b", "cp1_conv_b",
                 "cp1_up_b", "cp2_down_b", "cp2_conv_b", "cp2_up_b",
                 "ln1_b", "ln2_b"]
    for n in vec_names:
        nz[n] = bool(np.any(np.asarray(inputs[n]) != 0.0))
    nz["ln1_g"] = not bool(np.all(np.asarray(inputs["ln1_g"]) == 1.0))
    nz["ln2_g"] = not bool(np.all(np.asarray(inputs["ln2_g"]) == 1.0))
    return nz


def make_in_maps(inputs, nz):
    import ml_dtypes
    f8np = ml_dtypes.float8_e4m3fn
    bfnp = ml_dtypes.bfloat16
    x = np.asarray(inputs["x"], dtype=np.float32)
    pos = np.asarray(inputs["pos"], dtype=np.float32)
    common = {}

    def pair_ct(w):
        # [C, M] f32 -> [CP, P, 2, M] fp8 with pair (p,j) = channel (2cp+j)*128+p
        Cd, M = w.shape
        v = (w * WS).reshape(CP, 2, P, M).transpose(0, 2, 1, 3)
        return np.ascontiguousarray(v.reshape(CP, P, 2 * M).astype(f8np))

    qkv_w = np.asarray(inputs["qkv_w"], np.float32)
    common["qkv_w"] = pair_ct(qkv_w)
    common["proj_w"] = pair_ct(np.asarray(inputs["proj_w"], np.float32))
    # ff_w1 fp8 DoubleRow layout: [g, p, cp, j, mtj, m], ct = 2*cp + j
    w1 = (np.asarray(inputs["ff_w1"], np.float32) * WS).reshape(2, 2, P, W1G, 8, P)
    common["ff_w1"] = np.ascontiguousarray(
        w1.transpose(3, 2, 0, 1, 4, 5).reshape(W1G, P, CT * 8 * P).astype(f8np))
    # ff_w2 fp8 DoubleRow layout: [g, p, mtpg, jj, m], mt = 2*mtp + jj
    w2 = (np.asarray(inputs["ff_w2"], np.float32) * WS).reshape(W1G, 4, 2, P, C)
    common["ff_w2"] = np.ascontiguousarray(
        w2.transpose(0, 3, 1, 2, 4).reshape(W1G, P, 8 * C).astype(f8np))
    for i in (1, 2):
        dw = np.asarray(inputs[f"cp{i}_down_w"], np.float32)  # [C, ADIM]
        # 96-col layout: col 32*dxi + a <- dw[:, a] (x-shift blocks)
        dw96 = np.zeros((C, 96), np.float32)
        for dxi in range(3):
            dw96[:, 32 * dxi:32 * dxi + ADIM] = dw
        common[f"cp{i}_down_w"] = pair_ct(dw96)
        cw = np.asarray(inputs[f"cp{i}_conv_w"], np.float32)  # [O,I,3,3,3]
        w96 = np.zeros((96, 9, ADIM), np.float32)
        for dxi in range(3):
            w96[32 * dxi:32 * dxi + ADIM] = cw[:, :, :, :, dxi].transpose(
                1, 2, 3, 0).reshape(ADIM, 9, ADIM)
        common[f"cp{i}_conv_w"] = np.ascontiguousarray(
            w96.reshape(96, 72).astype(bfnp))
        common[f"cp{i}_up_w"] = np.ascontiguousarray(
            np.asarray(inputs[f"cp{i}_up_w"], np.float32).astype(bfnp))
    for n, flag in nz.items():
        if flag:
            common[n] = np.ascontiguousarray(
                np.asarray(inputs[n], np.float32)).reshape(-1, 1)
    in_maps = []
    for c in range(NCORES):
        m = dict(common)
        m["x"] = np.ascontiguousarray(
            x[c * BPC:(c + 1) * BPC].transpose(2, 0, 1).reshape(C, TOK))
        m["pos"] = np.ascontiguousarray(
            pos[c * BPC:(c + 1) * BPC].transpose(2, 0, 1).reshape(C, TOK))
        in_maps.append(m)
    return in_maps


def kernel(**inputs):
    nz = input_flags(inputs)
    nz_key = tuple(sorted((k, v) for k, v in nz.items()))
    nc = _get_compiled(nz_key, nz)
    in_maps = make_in_maps(inputs, nz)
    res = run_bass_kernel_spmd(nc, in_maps, core_ids=list(range(NCORES)))
    out = np.concatenate(
        [res.results[c]["out"].reshape(C, BPC, N).transpose(1, 2, 0)
         for c in range(NCORES)], axis=0)
    return np.ascontiguousarray(out.astype(np.float32))


if __name__ == "__main__":
    # quick self-build check (no run)
    nc = bacc.Bacc("TRN2", target_bir_lowering=False, debug=False, num_devices=NCORES)
    build(nc, {})
    nc.compile()
    print("built + compiled OK; instructions:",
          sum(len(bb.instructions) for bb in nc.main_func.blocks))
